# revision 90
# baseline (speedup 1.0000x reference)
"""Trainium2 Bass kernel for nn_Attention_87668872446719.

Patch-attention module: v = Conv3x3(x); xe = PatchEmbed(x); q,k = Linear(xe);
attn = softmax(q k^T / sqrt(hd)); out = Fold(attn @ Unfold(v)); out = Conv1x1(out).

Identity used (validated numerically): the unfold/attn/fold pipeline equals,
per channel c with head h = c // 32:
    folded[c, patch n, off] = sum_m attn[h, n, m] * v[c, patch m, off]

Sharding (8 cores, no collectives): core = (image b in 0..3, half s in 0..1).
s splits every 16x16 patch into its top/bottom 8 rows (off = ki*16+kj with
ki in [8s, 8s+8)), so the 1x1 proj stays pixel-local per core and each core
writes disjoint output rows.

Per core on device (all matmuls bf16, f32 PSUM accumulation):
  1. v conv first (warms the PE), TRANSPOSED output: lhsT = im2col slice
     [27, m-chunk] (pixel cols ordered o-major), rhs = wvT[27, 256] ->
     psum[m, 256 c] per o -> evict into VT[mc] = [m, (o, c)] bf16.
     V never leaves SBUF and needs no partition shuffle.
  2. xeT[256,196] = patch embed; qT/kT[32,196] per head (q pre-scaled)
  3. S[n,m] per head -> softmax; 1/rowsum folded into the bf16 cast of A;
     A transposed to AT[m, n] via PE (chunks of 98)
  4. stage E (F^T form): for each c: psum[off(128), n(196)] accumulated
     over m-chunks with lhsT = VT[:, c::256] (o-strided), rhs = AT
     -> fsb[off, (c, n)] -> fdram[off, c, n] (bf16, 12.5KB contiguous
     writes per off-row; the o<->c scatter cost is paid on the read side
     where it overlaps stage E + proj)
  5. proj: out[oc, (off n)] = projw @ F read back as [c, (32 off, n)]
     tiles; bf16 output (host upcasts to f32)
"""
from contextlib import ExitStack

import numpy as np
import ml_dtypes

import concourse.bass as bass
import concourse.tile as tile
from concourse import bacc, mybir
from concourse.bass_utils import run_bass_kernel_spmd

B, CIN, H, W = 4, 3, 224, 224
P = 16
DIM = 256
HEADS = 8
Hp = Wp = 14
N = Hp * Wp            # 196 patches
HD = DIM // HEADS      # 32
KI = 8                 # patch rows per core
OFF = KI * P           # 128 within-patch pixels per core
NPIX = N * OFF         # 25088 pixels per core
MCH = 98               # m-chunk (2 chunks of 98)
NCH = 98               # n-chunk for softmax/transposes
Q = 49                 # conv m-block (4 blocks of 49 m per cc)
BF = mybir.dt.bfloat16
F32 = mybir.dt.float32
AFT = mybir.ActivationFunctionType
AX = mybir.AxisListType.X

_CACHE = {}


def _build():
    nc = bacc.Bacc("TRN2", target_bir_lowering=False, debug=False)

    xcol_d = nc.declare_dram_parameter("xcol", [108, 8192], BF, isOutput=False)
    patches_d = nc.declare_dram_parameter("patches", [128, 6, N], BF, isOutput=False)
    pwT_d = nc.declare_dram_parameter("pwT", [128, 6, DIM], BF, isOutput=False)
    qkwT_d = nc.declare_dram_parameter("qkwT", [128, 2, 2 * DIM], BF, isOutput=False)
    wvT_d = nc.declare_dram_parameter("wvT", [108, 1024], BF, isOutput=False)
    projwT_d = nc.declare_dram_parameter("projwT", [128, 2, DIM], BF, isOutput=False)
    pbias_d = nc.declare_dram_parameter("pbias", [128, 2], F32, isOutput=False)
    obias_d = nc.declare_dram_parameter("obias", [128, 2], F32, isOutput=False)
    ident_d = nc.declare_dram_parameter("ident", [NCH, NCH], BF, isOutput=False)
    out_d = nc.declare_dram_parameter("out", [DIM, NPIX], BF, isOutput=True)

    fdram = nc.dram_tensor("fdram", [OFF, DIM, N], BF)       # [off, c, n]

    with tile.TileContext(nc) as tc, ExitStack() as ctx:
        const = ctx.enter_context(tc.tile_pool(name="const", bufs=1))
        stat = ctx.enter_context(tc.tile_pool(name="stat", bufs=4))
        sb = ctx.enter_context(tc.tile_pool(name="sb", bufs=2))
        atp = ctx.enter_context(tc.tile_pool(name="atp", bufs=1))
        pP = ctx.enter_context(tc.tile_pool(name="pP", bufs=2, space="PSUM"))
        pA = ctx.enter_context(tc.tile_pool(name="pA", bufs=3, space="PSUM"))
        pfr1 = ctx.enter_context(tc.tile_pool(name="pfr1", bufs=1))
        pfr0 = ctx.enter_context(tc.tile_pool(name="pfr0", bufs=2))
        vctx = ctx.enter_context(ExitStack())
        vtp = vctx.enter_context(tc.tile_pool(name="vtp", bufs=1))
        GW = 32 * N  # 6272 cols per og chunk

        def fr_load_cc1(og):
            fr = pfr1.tile([128, GW], BF, tag="fr1", name="fr")
            src = fdram[og * 32:(og + 1) * 32, 128:256, :].rearrange(
                "o c n -> c o n")
            nc.sync.dma_start(
                fr[:].rearrange("c (o n) -> c o n", n=N), src)
            return fr

        # ---- constants (spread across issue queues; xcol gates conv and
        # is issued first, on its own queue) ----
        qrot = [nc.scalar, nc.sync]

        def cload(shape, dt, dram, tag, qi=[0]):
            t = const.tile(shape, dt, tag=tag, name=tag)
            q = qrot[qi[0] % 2]
            qi[0] += 1
            q.dma_start(t[:], dram[:])
            return t

        with tc.high_priority():
            wvT_t = cload([108, 1024], BF, wvT_d, "c_wvT")
        patches_t = cload([128, 6, N], BF, patches_d, "c_patches")
        pwT_t = cload([128, 6, DIM], BF, pwT_d, "c_pwT")
        qkwT_t = cload([128, 2, 2 * DIM], BF, qkwT_d, "c_qkwT")
        projwT_t = cload([128, 2, DIM], BF, projwT_d, "c_projwT")
        pbias_t = cload([128, 2], F32, pbias_d, "c_pbias")
        obias_t = cload([128, 2], F32, obias_d, "c_obias")
        ident_t = cload([NCH, NCH], BF, ident_d, "c_ident")

        # VT[mc]: [128, (128 off, 256 c)] bf16, partition = m (m padded
        # to 256 with zero rows so stage E runs K=128)
        VT = [vtp.tile([128, OFF * DIM], BF, tag="vt%d" % mc,
                       name="vt%d" % mc) for mc in range(2)]

        ev_flip = [0]

        def evict(dst, src, scale=None, bias=None):
            """PSUM -> SBUF eviction alternating DVE / ACT."""
            e = ev_flip[0] = 1 - ev_flip[0]
            if scale is not None:
                if e:
                    nc.vector.tensor_scalar_mul(dst, src, scale)
                else:
                    nc.scalar.activation(dst, src, AFT.Copy, scale=scale)
            elif bias is not None:
                if e:
                    nc.vector.tensor_scalar_add(dst, src, bias)
                else:
                    nc.scalar.activation(dst, src, AFT.Identity, bias=bias)
            else:
                if e:
                    nc.vector.tensor_copy(dst, src)
                else:
                    nc.scalar.copy(dst, src)

        # ---- stage D first (warms PE early): v conv, transposed out ----
        # K packed to 108 = 4 off x 27 k (HAM duty 84%): lhsT = xcol
        # [108, m-chunk 128] (cols (o-quad, m_pad 256)); rhs = wvT
        # block-diagonal [108, (2 chalf, 4 off, 128 c)]; 2 MMs of N=512
        # per (o-quad, mc) share one LDWEIGHTS.
        with tc.tile_pool(name="px", bufs=1) as px:
            xcol_t = px.tile([108, 8192], BF, tag="xcol", name="xcol")
            with tc.high_priority():
                for i4 in range(4):
                    q = nc.sync if i4 % 2 == 0 else nc.scalar
                    q.dma_start(xcol_t[:, i4 * 2048:(i4 + 1) * 2048],
                                xcol_d[:, i4 * 2048:(i4 + 1) * 2048])
            for o4 in range(32):      # 4 off per psum tile
                for mc in range(2):
                    ps = pA.tile([128, 1024], F32, tag="mm", name="psc")
                    for chalf in range(2):
                        nc.tensor.matmul(
                            ps[:, chalf * 512:(chalf + 1) * 512],
                            xcol_t[:, o4 * 256 + mc * 128:
                                   o4 * 256 + mc * 128 + 128],
                            wvT_t[:, chalf * 512:(chalf + 1) * 512],
                            start=True, stop=True)
                    # psum (chalf, off, c) -> VT (off, chalf, c):
                    # one engine per chalf, in parallel
                    vtv = VT[mc][:, o4 * 1024:(o4 + 1) * 1024].rearrange(
                        "m (o h c) -> m h o c", o=4, h=2)
                    for h2 in range(2):
                        s2 = ps[:, h2 * 512:(h2 + 1) * 512].rearrange(
                            "m (o c) -> m o c", o=4)
                        if h2 == 0:
                            nc.vector.tensor_copy(vtv[:, h2], s2)
                        else:
                            nc.scalar.copy(vtv[:, h2], s2)

        # ---- stage A: xeT[c, n] = patch embed (transposed) ----
        xeT = []
        for cc in range(2):
            ps = pP.tile([128, N], F32, tag="sm", name="pse")
            for kc in range(6):
                nc.tensor.matmul(
                    ps[:], pwT_t[:, kc, cc * 128:(cc + 1) * 128],
                    patches_t[:, kc, :], start=(kc == 0), stop=(kc == 5))
            xt = sb.tile([128, N], BF, tag="xeT%d" % cc, name="xeT")
            nc.vector.tensor_scalar_add(xt[:], ps[:], pbias_t[:, cc:cc + 1])
            xeT.append(xt)

        # ---- stage B/C: per-head q/k, scores, softmax, AT ----
        # (emitted per-head, software-pipelined into stage E)
        AT = {}     # AT[h][mc] : [128, 196] bf16 (A^T, normalized)

        def phase1_head(h):
            qT = sb.tile([HD, N], BF, tag="qT", name="qT")
            kT = sb.tile([HD, N], BF, tag="kT", name="kT")
            for dst, joff in ((qT, h * HD), (kT, DIM + h * HD)):
                ps = pP.tile([HD, N], F32, tag="sm", name="psq")
                for cc in range(2):
                    nc.tensor.matmul(
                        ps[:], qkwT_t[:, cc, joff:joff + HD], xeT[cc][:],
                        start=(cc == 0), stop=(cc == 1))
                nc.scalar.copy(dst[:], ps[:])

            Ah = []
            for nci in range(2):
                nb = nci * NCH
                ps = pP.tile([NCH, N], F32, tag="sm", name="pss")
                nc.tensor.matmul(ps[:], qT[:, nb:nb + NCH], kT[:],
                                 start=True, stop=True)
                mx = stat.tile([NCH, 1], F32, tag="mx", name="mx")
                nc.vector.reduce_max(mx[:], ps[:], axis=AX, negate=True)
                ex = sb.tile([NCH, N], BF, tag="ex", name="ex")
                nc.scalar.activation(ex[:], ps[:], AFT.Exp, bias=mx[:])
                sm = stat.tile([NCH, 1], F32, tag="smm", name="smm")
                nc.vector.reduce_sum(sm[:], ex[:], axis=AX)
                rc = stat.tile([NCH, 1], F32, tag="rc", name="rc")
                nc.vector.reciprocal(rc[:], sm[:])
                ab = sb.tile([NCH, 256], BF, tag="ab", name="ab")
                nc.vector.tensor_scalar_mul(ab[:, :N], ex[:], rc[:])
                nc.vector.memset(ab[:, N:], 0.0)
                Ah.append(ab)

            ATh = []
            for mc in range(2):
                at = atp.tile([128, N], BF, tag="at%d_%d" % (mc, h), name="at")
                mb = mc * 128
                for nci in range(2):
                    nb = nci * NCH
                    pt = pP.tile([128, NCH], BF, tag="sm", name="pst")
                    nc.tensor.transpose(pt[:], Ah[nci][:, mb:mb + 128],
                                        ident_t[:])
                    evict(at[:, nb:nb + NCH], pt[:])
                ATh.append(at)
            AT[h] = ATh

        # ---- stage E (F^T form): psum[off, n] per c, evict to fsb ----
        # VT free layout is (q32, h2, o4, c128); lhsT for global c is a
        # 3D strided slice.  Head order interleaves cc0/cc1 so the
        # scattered fdramA writes get two-head time budgets.
        VTv = [VT[mc].rearrange("m (o c) -> m o c", c=DIM) for mc in range(2)]
        fr1_pend = {}
        fr0_pend = {}
        HORDER = (4, 5, 6, 7, 0, 1, 2, 3)
        phase1_head(HORDER[0])
        phase1_head(HORDER[1])
        with tc.tile_pool(name="fsp", bufs=2) as fsp:
            for hi, h in enumerate(HORDER):
                if hi + 2 < HEADS:
                    phase1_head(HORDER[hi + 2])
                if hi == 4:
                    # cc1 F rows complete: prefetch first cc1 read and
                    # allocate early cc0 fr tiles (filled per head below)
                    fr1_pend[0] = fr_load_cc1(0)
                    fr0_pend[0] = pfr0.tile([128, GW], BF, tag="fr0e",
                                            name="fr", bufs=1)
                for half in range(2):
                    fsb = fsp.tile([128, 16 * N], BF, tag="fsb", name="fsb")
                    for jh in range(4):   # groups of 4 c
                        jj = half * 4 + jh
                        ps = pA.tile([128, 1024], F32, tag="mm", name="psf")
                        for j2 in range(4):
                            cg = h * 32 + jj * 4 + j2
                            o0 = (j2 // 2) * 512 + (j2 % 2) * N
                            for mc in range(2):
                                nc.tensor.matmul(
                                    ps[:, o0:o0 + N],
                                    VTv[mc][:, :, cg],
                                    AT[h][mc][:],
                                    start=(mc == 0), stop=(mc == 1))
                        src = ps[:].rearrange(
                            "p (b x) -> p b x", b=2)[:, :, :2 * N]
                        dst = fsb[:, jh * 4 * N:(jh + 1) * 4 * N].rearrange(
                            "p (b x) -> p b x", b=2)
                        evict(dst, src)
                    c0 = h * HD + half * 16
                    fd = fdram[:, c0:c0 + 16, :]
                    nc.sync.dma_start(
                        fd, fsb[:].rearrange("o (c n) -> o c n", n=N))
                if h < 4:
                    # this cc0 head's F rows are final: pull them into
                    # the early og0 fr tile right away
                    dst = fr0_pend[0][h * HD:(h + 1) * HD, :].rearrange(
                        "c (o n) -> c o n", n=N)
                    nc.sync.dma_start(
                        dst, fdram[0:32, h * HD:(h + 1) * HD, :].rearrange(
                            "o c n -> c o n"))

        # ---- stage F: proj from fdram, bf16 out ----
        # fr tiles [c(128), (16 off, 196 n)] per (og16, cc).  pfr lives
        # alongside VT (og16 keeps it small) so the first reads overlap
        # the stage E tail; bufs=2 keeps reads 2 chunks ahead of the
        # matmuls.  Out cols remain (off, n) order.
        vctx.close()   # free VT before proj pools allocate
        posb = ctx.enter_context(tc.tile_pool(name="posb", bufs=3))

        def fr_load_cc0(og):
            fr = pfr0.tile([128, GW], BF, tag="fr0", name="fr", bufs=1)
            src = fdram[og * 32:(og + 1) * 32, 0:128, :].rearrange(
                "o c n -> c o n")
            nc.sync.dma_start(
                fr[:].rearrange("c (o n) -> c o n", n=N), src)
            return fr

        fr0_pend[1] = fr_load_cc0(1)
        fr1_pend[1] = fr_load_cc1(1)
        for og in range(4):
            if og + 2 < 4:
                fr0_pend[og + 2] = fr_load_cc0(og + 2)
                fr1_pend[og + 2] = fr_load_cc1(og + 2)
            frs = [fr0_pend[og], fr1_pend[og]]
            for occ in range(2):
                ot = posb.tile([128, GW], BF, tag="osb", name="osb")
                for t6 in range(7):
                    w = 1024 if t6 < 6 else 128
                    ps = pA.tile([128, 1024], F32, tag="mm", name="psp")
                    for half in range((w + 511) // 512):
                        b0 = t6 * 1024 + half * 512
                        bw = min(512, w - half * 512)
                        for cc in range(2):
                            nc.tensor.matmul(
                                ps[:, half * 512:half * 512 + bw],
                                projwT_t[:, cc, occ * 128:(occ + 1) * 128],
                                frs[cc][:, b0:b0 + bw],
                                start=(cc == 0), stop=(cc == 1))
                    evict(ot[:, t6 * 1024:t6 * 1024 + w], ps[:, :w],
                          bias=obias_t[:, occ:occ + 1])
                nc.sync.dma_start(
                    out_d[occ * 128:(occ + 1) * 128,
                          og * GW:(og + 1) * GW], ot[:])

    nc.compile()
    return nc


def _host_prep(inputs):
    """Returns per-core in_maps."""
    x = np.asarray(inputs["x"], np.float32)
    patch_w = np.asarray(inputs["patch_w"], np.float32)
    patch_b = np.asarray(inputs["patch_b"], np.float32)
    qk_w = np.asarray(inputs["qk_w"], np.float32)
    v_w = np.asarray(inputs["v_w"], np.float32)
    v_b = np.asarray(inputs["v_b"], np.float32)
    proj_w = np.asarray(inputs["proj_w"], np.float32).reshape(DIM, DIM)
    proj_b = np.asarray(inputs["proj_b"], np.float32)

    bf = ml_dtypes.bfloat16
    pw = patch_w.reshape(DIM, CIN * P * P)                     # [256, 768]
    pwT = pw.T.reshape(6, 128, DIM).transpose(1, 0, 2)         # [128, 6, 256]
    qkw = qk_w.copy()
    qkw[:DIM] *= HD ** -0.5                                    # fold attn scale
    qkwT = qkw.T.reshape(2, 128, 2 * DIM).transpose(1, 0, 2)   # [128, 2, 512]
    wvT = v_w.reshape(DIM, 27).T                               # [27, 256]
    # block-diagonal over 4 off-slices: [(4 o', 27 k), (2 ch, 4 o'', 128 c)]
    wvT4 = np.zeros((108, 1024), np.float32)
    for op in range(4):
        for ch in range(2):
            wvT4[op * 27:(op + 1) * 27,
                 ch * 512 + op * 128:ch * 512 + op * 128 + 128] = \
                wvT[:, ch * 128:(ch + 1) * 128]
    projwT = proj_w.T.reshape(2, 128, DIM).transpose(1, 0, 2)  # [128, 2, 256]
    pbias = patch_b.reshape(2, 128).T.copy()                   # [128, 2]
    obias = (proj_w @ v_b + proj_b).reshape(2, 128).T.copy()   # [128, 2]

    shared = {
        "pwT": pwT.astype(bf), "qkwT": qkwT.astype(bf),
        "wvT": wvT4.astype(bf), "projwT": projwT.astype(bf),
        "pbias": pbias.astype(np.float32), "obias": obias.astype(np.float32),
        "ident": np.eye(NCH, dtype=bf),
    }

    in_maps = []
    for b in range(B):
        # patches: [768, 196] part order (ci, ki, kj) -> [128, 6, 196]
        p4 = x[b].reshape(CIN, Hp, P, Wp, P).transpose(0, 2, 4, 1, 3)
        patches = p4.reshape(CIN * P * P, N).reshape(6, 128, N)
        patches = patches.transpose(1, 0, 2).astype(bf)
        xpad = np.zeros((CIN, H + 2, W + 2), np.float32)
        xpad[:, 1:-1, 1:-1] = x[b]
        for s in range(2):
            cols = np.empty((CIN, 3, 3, Hp, Wp, KI, P), np.float32)
            for dy in range(3):
                for dx in range(3):
                    view = xpad[:, dy:dy + H, dx:dx + W]
                    v4 = view.reshape(CIN, Hp, P, Wp, P)[:, :, 8 * s:8 * s + 8]
                    cols[:, dy, dx] = v4.transpose(0, 1, 3, 2, 4)
            # [27, m, off] -> [(4 o', 27 k), (32 q, 256 m_pad)]
            xc = cols.reshape(27, N, OFF).transpose(0, 2, 1)   # [27, off, m]
            tmp = np.zeros((27, OFF, 256), np.float32)
            tmp[:, :, :N] = xc
            xcol = tmp.reshape(27, 32, 4, 256).transpose(2, 0, 1, 3)
            xcol = xcol.reshape(108, 8192).astype(bf)
            in_maps.append(dict(shared, xcol=xcol, patches=patches))
    return in_maps


def kernel(**inputs):
    if "nc" not in _CACHE:
        _CACHE["nc"] = _build()
    nc = _CACHE["nc"]
    in_maps = _host_prep(inputs)
    res = run_bass_kernel_spmd(nc, in_maps, core_ids=list(range(8)))
    out = np.zeros((B, DIM, H, W), np.float32)
    ov = out.reshape(B, DIM, Hp, P, Wp, P)
    for i, r in enumerate(res.results):
        b, s = divmod(i, 2)
        # out cols = (off, n) = (ki, kj, hp, wp)
        o = np.asarray(r["out"], dtype=np.float32)
        o = o.reshape(DIM, KI, P, Hp, Wp)
        ov[b, :, :, 8 * s:8 * s + 8, :, :] = o.transpose(0, 3, 1, 4, 2)
    return out


# revision 91
# speedup vs baseline: 1.0517x; 1.0517x over previous
"""Trainium2 Bass kernel for nn_Attention_87668872446719.

Patch-attention module: v = Conv3x3(x); xe = PatchEmbed(x); q,k = Linear(xe);
attn = softmax(q k^T / sqrt(hd)); out = Fold(attn @ Unfold(v)); out = Conv1x1(out).

Identity used (validated numerically): the unfold/attn/fold pipeline equals,
per channel c with head h = c // 32:
    folded[c, patch n, off] = sum_m attn[h, n, m] * v[c, patch m, off]

Sharding (8 cores, no collectives): core = (image b in 0..3, half s in 0..1).
s splits every 16x16 patch into its top/bottom 8 rows (off = ki*16+kj with
ki in [8s, 8s+8)), so the 1x1 proj stays pixel-local per core and each core
writes disjoint output rows.

Per core on device (all matmuls bf16, f32 PSUM accumulation):
  1. v conv first (warms the PE), TRANSPOSED output: lhsT = im2col slice
     [27, m-chunk] (pixel cols ordered o-major), rhs = wvT[27, 256] ->
     psum[m, 256 c] per o -> evict into VT[mc] = [m, (o, c)] bf16.
     V never leaves SBUF and needs no partition shuffle.
  2. xeT[256,196] = patch embed; qT/kT[32,196] per head (q pre-scaled)
  3. S[n,m] per head -> softmax; 1/rowsum folded into the bf16 cast of A;
     A transposed to AT[m, n] via PE (chunks of 98)
  4. stage E (F^T form): for each c: psum[off(128), n(196)] accumulated
     over m-chunks with lhsT = VT[:, c::256] (o-strided), rhs = AT
     -> fsb[off, (c, n)] -> fdram[off, c, n] (bf16, 12.5KB contiguous
     writes per off-row; the o<->c scatter cost is paid on the read side
     where it overlaps stage E + proj)
  5. proj: out[oc, (off n)] = projw @ F read back as [c, (32 off, n)]
     tiles; bf16 output (host upcasts to f32)
"""
from contextlib import ExitStack

import numpy as np
import ml_dtypes

import concourse.bass as bass
import concourse.tile as tile
from concourse import bacc, mybir
from concourse.bass_utils import run_bass_kernel_spmd

B, CIN, H, W = 4, 3, 224, 224
P = 16
DIM = 256
HEADS = 8
Hp = Wp = 14
N = Hp * Wp            # 196 patches
HD = DIM // HEADS      # 32
KI = 8                 # patch rows per core
OFF = KI * P           # 128 within-patch pixels per core
NPIX = N * OFF         # 25088 pixels per core
MCH = 98               # m-chunk (2 chunks of 98)
NCH = 98               # n-chunk for softmax/transposes
Q = 49                 # conv m-block (4 blocks of 49 m per cc)
BF = mybir.dt.bfloat16
F32 = mybir.dt.float32
AFT = mybir.ActivationFunctionType
AX = mybir.AxisListType.X

_CACHE = {}


def _build():
    nc = bacc.Bacc("TRN2", target_bir_lowering=False, debug=False)

    xcol_d = nc.declare_dram_parameter("xcol", [108, 8192], BF, isOutput=False)
    patches_d = nc.declare_dram_parameter("patches", [128, 6, N], BF, isOutput=False)
    pwT_d = nc.declare_dram_parameter("pwT", [128, 6, DIM], BF, isOutput=False)
    qkwT_d = nc.declare_dram_parameter("qkwT", [128, 2, 2 * DIM], BF, isOutput=False)
    wvT_d = nc.declare_dram_parameter("wvT", [108, 1024], BF, isOutput=False)
    projwT_d = nc.declare_dram_parameter("projwT", [128, 2, DIM], BF, isOutput=False)
    pbias_d = nc.declare_dram_parameter("pbias", [128, 2], F32, isOutput=False)
    obias_d = nc.declare_dram_parameter("obias", [128, 2], F32, isOutput=False)
    ident_d = nc.declare_dram_parameter("ident", [NCH, NCH], BF, isOutput=False)
    out_d = nc.declare_dram_parameter("out", [DIM, NPIX], BF, isOutput=True)

    fdram = nc.dram_tensor("fdram", [OFF, DIM, N], BF)       # [off, c, n]

    with tile.TileContext(nc) as tc, ExitStack() as ctx:
        const = ctx.enter_context(tc.tile_pool(name="const", bufs=1))
        stat = ctx.enter_context(tc.tile_pool(name="stat", bufs=4))
        sb = ctx.enter_context(tc.tile_pool(name="sb", bufs=2))
        atp = ctx.enter_context(tc.tile_pool(name="atp", bufs=1))
        pP = ctx.enter_context(tc.tile_pool(name="pP", bufs=2, space="PSUM"))
        pA = ctx.enter_context(tc.tile_pool(name="pA", bufs=3, space="PSUM"))
        pfr1 = ctx.enter_context(tc.tile_pool(name="pfr1", bufs=2))
        vctx = ctx.enter_context(ExitStack())
        vtp = vctx.enter_context(tc.tile_pool(name="vtp", bufs=1))
        GW = 32 * N  # 6272 cols per og chunk

        def fr_load_cc1(og):
            fr = pfr1.tile([128, GW], BF, tag="fr1", name="fr")
            src = fdram[og * 32:(og + 1) * 32, 128:256, :].rearrange(
                "o c n -> c o n")
            nc.sync.dma_start(
                fr[:].rearrange("c (o n) -> c o n", n=N), src)
            return fr

        # ---- constants (spread across issue queues; xcol gates conv and
        # is issued first, on its own queue) ----
        qrot = [nc.scalar, nc.sync]

        def cload(shape, dt, dram, tag, qi=[0]):
            t = const.tile(shape, dt, tag=tag, name=tag)
            q = qrot[qi[0] % 2]
            qi[0] += 1
            q.dma_start(t[:], dram[:])
            return t

        with tc.high_priority():
            wvT_t = cload([108, 1024], BF, wvT_d, "c_wvT")
        patches_t = cload([128, 6, N], BF, patches_d, "c_patches")
        pwT_t = cload([128, 6, DIM], BF, pwT_d, "c_pwT")
        qkwT_t = cload([128, 2, 2 * DIM], BF, qkwT_d, "c_qkwT")
        projwT_t = cload([128, 2, DIM], BF, projwT_d, "c_projwT")
        pbias_t = cload([128, 2], F32, pbias_d, "c_pbias")
        obias_t = cload([128, 2], F32, obias_d, "c_obias")
        ident_t = cload([NCH, NCH], BF, ident_d, "c_ident")

        # VT[mc]: [128, (128 off, 256 c)] bf16, partition = m (m padded
        # to 256 with zero rows so stage E runs K=128)
        VT = [vtp.tile([128, OFF * DIM], BF, tag="vt%d" % mc,
                       name="vt%d" % mc) for mc in range(2)]

        ev_flip = [0]

        def evict(dst, src, scale=None, bias=None):
            """PSUM -> SBUF eviction alternating DVE / ACT."""
            e = ev_flip[0] = 1 - ev_flip[0]
            if scale is not None:
                if e:
                    nc.vector.tensor_scalar_mul(dst, src, scale)
                else:
                    nc.scalar.activation(dst, src, AFT.Copy, scale=scale)
            elif bias is not None:
                if e:
                    nc.vector.tensor_scalar_add(dst, src, bias)
                else:
                    nc.scalar.activation(dst, src, AFT.Identity, bias=bias)
            else:
                if e:
                    nc.vector.tensor_copy(dst, src)
                else:
                    nc.scalar.copy(dst, src)

        # ---- stage D first (warms PE early): v conv, transposed out ----
        # K packed to 108 = 4 off x 27 k (HAM duty 84%): lhsT = xcol
        # [108, m-chunk 128] (cols (o-quad, m_pad 256)); rhs = wvT
        # block-diagonal [108, (2 chalf, 4 off, 128 c)]; 2 MMs of N=512
        # per (o-quad, mc) share one LDWEIGHTS.
        with tc.tile_pool(name="px", bufs=1) as px:
            xcol_t = px.tile([108, 8192], BF, tag="xcol", name="xcol")
            with tc.high_priority():
                for i4 in range(4):
                    q = nc.sync if i4 % 2 == 0 else nc.scalar
                    q.dma_start(xcol_t[:, i4 * 2048:(i4 + 1) * 2048],
                                xcol_d[:, i4 * 2048:(i4 + 1) * 2048])
            for o4 in range(32):      # 4 off per psum tile
                for mc in range(2):
                    ps = pA.tile([128, 1024], F32, tag="mm", name="psc")
                    for chalf in range(2):
                        nc.tensor.matmul(
                            ps[:, chalf * 512:(chalf + 1) * 512],
                            xcol_t[:, o4 * 256 + mc * 128:
                                   o4 * 256 + mc * 128 + 128],
                            wvT_t[:, chalf * 512:(chalf + 1) * 512],
                            start=True, stop=True)
                    # psum (chalf, off, c) -> VT (off, chalf, c):
                    # one engine per chalf, in parallel
                    vtv = VT[mc][:, o4 * 1024:(o4 + 1) * 1024].rearrange(
                        "m (o h c) -> m h o c", o=4, h=2)
                    for h2 in range(2):
                        s2 = ps[:, h2 * 512:(h2 + 1) * 512].rearrange(
                            "m (o c) -> m o c", o=4)
                        if h2 == 0:
                            nc.vector.tensor_copy(vtv[:, h2], s2)
                        else:
                            nc.scalar.copy(vtv[:, h2], s2)

        # ---- stage A: xeT[c, n] = patch embed (transposed) ----
        xeT = []
        for cc in range(2):
            ps = pP.tile([128, N], F32, tag="sm", name="pse")
            for kc in range(6):
                nc.tensor.matmul(
                    ps[:], pwT_t[:, kc, cc * 128:(cc + 1) * 128],
                    patches_t[:, kc, :], start=(kc == 0), stop=(kc == 5))
            xt = sb.tile([128, N], BF, tag="xeT%d" % cc, name="xeT")
            nc.vector.tensor_scalar_add(xt[:], ps[:], pbias_t[:, cc:cc + 1])
            xeT.append(xt)

        # ---- stage B/C: per-head q/k, scores, softmax, AT ----
        # (emitted per-head, software-pipelined into stage E)
        AT = {}     # AT[h][mc] : [128, 196] bf16 (A^T, normalized)

        def phase1_head(h):
            qT = sb.tile([HD, N], BF, tag="qT", name="qT")
            kT = sb.tile([HD, N], BF, tag="kT", name="kT")
            for dst, joff in ((qT, h * HD), (kT, DIM + h * HD)):
                ps = pP.tile([HD, N], F32, tag="sm", name="psq")
                for cc in range(2):
                    nc.tensor.matmul(
                        ps[:], qkwT_t[:, cc, joff:joff + HD], xeT[cc][:],
                        start=(cc == 0), stop=(cc == 1))
                nc.scalar.copy(dst[:], ps[:])

            Ah = []
            for nci in range(2):
                nb = nci * NCH
                ps = pP.tile([NCH, N], F32, tag="sm", name="pss")
                nc.tensor.matmul(ps[:], qT[:, nb:nb + NCH], kT[:],
                                 start=True, stop=True)
                mx = stat.tile([NCH, 1], F32, tag="mx", name="mx")
                nc.vector.reduce_max(mx[:], ps[:], axis=AX, negate=True)
                ex = sb.tile([NCH, N], BF, tag="ex", name="ex")
                nc.scalar.activation(ex[:], ps[:], AFT.Exp, bias=mx[:])
                sm = stat.tile([NCH, 1], F32, tag="smm", name="smm")
                nc.vector.reduce_sum(sm[:], ex[:], axis=AX)
                rc = stat.tile([NCH, 1], F32, tag="rc", name="rc")
                nc.vector.reciprocal(rc[:], sm[:])
                ab = sb.tile([NCH, 256], BF, tag="ab", name="ab")
                nc.vector.tensor_scalar_mul(ab[:, :N], ex[:], rc[:])
                nc.vector.memset(ab[:, N:], 0.0)
                Ah.append(ab)

            ATh = []
            for mc in range(2):
                at = atp.tile([128, N], BF, tag="at%d_%d" % (mc, h), name="at")
                mb = mc * 128
                for nci in range(2):
                    nb = nci * NCH
                    pt = pP.tile([128, NCH], BF, tag="sm", name="pst")
                    nc.tensor.transpose(pt[:], Ah[nci][:, mb:mb + 128],
                                        ident_t[:])
                    evict(at[:, nb:nb + NCH], pt[:])
                ATh.append(at)
            AT[h] = ATh

        # ---- stage E (F^T form): psum[off, n] per c, evict to fsb ----
        # VT free layout is (q32, h2, o4, c128); lhsT for global c is a
        # 3D strided slice.  Head order interleaves cc0/cc1 so the
        # scattered fdramA writes get two-head time budgets.
        VTv = [VT[mc].rearrange("m (o c) -> m o c", c=DIM) for mc in range(2)]
        fr1_pend = {}
        HORDER = (4, 5, 6, 7, 0, 1, 2, 3)
        phase1_head(HORDER[0])
        phase1_head(HORDER[1])
        with tc.tile_pool(name="fsp", bufs=2) as fsp:
            for hi, h in enumerate(HORDER):
                if hi + 2 < HEADS:
                    phase1_head(HORDER[hi + 2])
                if hi == 4:
                    # cc1 F rows complete: prefetch cc1 reads under the
                    # cc0 heads' compute
                    fr1_pend[0] = fr_load_cc1(0)
                    fr1_pend[1] = fr_load_cc1(1)
                fsb = fsp.tile([128, 32 * N], BF, tag="fsb", name="fsb")
                for jj in range(8):   # groups of 4 c
                    ps = pA.tile([128, 1024], F32, tag="mm", name="psf")
                    for j2 in range(4):
                        cg = h * 32 + jj * 4 + j2
                        o0 = (j2 // 2) * 512 + (j2 % 2) * N
                        for mc in range(2):
                            nc.tensor.matmul(
                                ps[:, o0:o0 + N],
                                VTv[mc][:, :, cg],
                                AT[h][mc][:],
                                start=(mc == 0), stop=(mc == 1))
                    for b2 in range(2):
                        s2 = ps[:, b2 * 512:b2 * 512 + 2 * N]
                        d2 = fsb[:, (jj * 4 + b2 * 2) * N:
                                 (jj * 4 + b2 * 2 + 2) * N]
                        if b2 == 0:
                            nc.vector.tensor_copy(d2, s2)
                        else:
                            nc.scalar.copy(d2, s2)
                fd = fdram[:, h * HD:(h + 1) * HD, :]
                nc.sync.dma_start(
                    fd, fsb[:].rearrange("o (c n) -> o c n", n=N))

        # ---- stage F: proj from fdram, bf16 out ----
        # fr tiles [c(128), (16 off, 196 n)] per (og16, cc).  pfr lives
        # alongside VT (og16 keeps it small) so the first reads overlap
        # the stage E tail; bufs=2 keeps reads 2 chunks ahead of the
        # matmuls.  Out cols remain (off, n) order.
        vctx.close()   # free VT before proj pools allocate
        pfr0 = ctx.enter_context(tc.tile_pool(name="pfr0", bufs=2))
        posb = ctx.enter_context(tc.tile_pool(name="posb", bufs=3))

        def fr_load_cc0(og):
            fr = pfr0.tile([128, GW], BF, tag="fr0", name="fr")
            src = fdram[og * 32:(og + 1) * 32, 0:128, :].rearrange(
                "o c n -> c o n")
            nc.sync.dma_start(
                fr[:].rearrange("c (o n) -> c o n", n=N), src)
            return fr

        fr0_pend = {0: fr_load_cc0(0), 1: fr_load_cc0(1)}
        for og in range(4):
            if og + 2 < 4:
                fr0_pend[og + 2] = fr_load_cc0(og + 2)
                fr1_pend[og + 2] = fr_load_cc1(og + 2)
            frs = [fr0_pend[og], fr1_pend[og]]
            for occ in range(2):
                ot = posb.tile([128, GW], BF, tag="osb", name="osb")
                for t6 in range(7):
                    w = 1024 if t6 < 6 else 128
                    ps = pA.tile([128, 1024], F32, tag="mm", name="psp")
                    for half in range((w + 511) // 512):
                        b0 = t6 * 1024 + half * 512
                        bw = min(512, w - half * 512)
                        for cc in range(2):
                            nc.tensor.matmul(
                                ps[:, half * 512:half * 512 + bw],
                                projwT_t[:, cc, occ * 128:(occ + 1) * 128],
                                frs[cc][:, b0:b0 + bw],
                                start=(cc == 0), stop=(cc == 1))
                    evict(ot[:, t6 * 1024:t6 * 1024 + w], ps[:, :w],
                          bias=obias_t[:, occ:occ + 1])
                nc.sync.dma_start(
                    out_d[occ * 128:(occ + 1) * 128,
                          og * GW:(og + 1) * GW], ot[:])

    nc.compile()
    return nc


def _host_prep(inputs):
    """Returns per-core in_maps."""
    x = np.asarray(inputs["x"], np.float32)
    patch_w = np.asarray(inputs["patch_w"], np.float32)
    patch_b = np.asarray(inputs["patch_b"], np.float32)
    qk_w = np.asarray(inputs["qk_w"], np.float32)
    v_w = np.asarray(inputs["v_w"], np.float32)
    v_b = np.asarray(inputs["v_b"], np.float32)
    proj_w = np.asarray(inputs["proj_w"], np.float32).reshape(DIM, DIM)
    proj_b = np.asarray(inputs["proj_b"], np.float32)

    bf = ml_dtypes.bfloat16
    pw = patch_w.reshape(DIM, CIN * P * P)                     # [256, 768]
    pwT = pw.T.reshape(6, 128, DIM).transpose(1, 0, 2)         # [128, 6, 256]
    qkw = qk_w.copy()
    qkw[:DIM] *= HD ** -0.5                                    # fold attn scale
    qkwT = qkw.T.reshape(2, 128, 2 * DIM).transpose(1, 0, 2)   # [128, 2, 512]
    wvT = v_w.reshape(DIM, 27).T                               # [27, 256]
    # block-diagonal over 4 off-slices: [(4 o', 27 k), (2 ch, 4 o'', 128 c)]
    wvT4 = np.zeros((108, 1024), np.float32)
    for op in range(4):
        for ch in range(2):
            wvT4[op * 27:(op + 1) * 27,
                 ch * 512 + op * 128:ch * 512 + op * 128 + 128] = \
                wvT[:, ch * 128:(ch + 1) * 128]
    projwT = proj_w.T.reshape(2, 128, DIM).transpose(1, 0, 2)  # [128, 2, 256]
    pbias = patch_b.reshape(2, 128).T.copy()                   # [128, 2]
    obias = (proj_w @ v_b + proj_b).reshape(2, 128).T.copy()   # [128, 2]

    shared = {
        "pwT": pwT.astype(bf), "qkwT": qkwT.astype(bf),
        "wvT": wvT4.astype(bf), "projwT": projwT.astype(bf),
        "pbias": pbias.astype(np.float32), "obias": obias.astype(np.float32),
        "ident": np.eye(NCH, dtype=bf),
    }

    in_maps = []
    for b in range(B):
        # patches: [768, 196] part order (ci, ki, kj) -> [128, 6, 196]
        p4 = x[b].reshape(CIN, Hp, P, Wp, P).transpose(0, 2, 4, 1, 3)
        patches = p4.reshape(CIN * P * P, N).reshape(6, 128, N)
        patches = patches.transpose(1, 0, 2).astype(bf)
        xpad = np.zeros((CIN, H + 2, W + 2), np.float32)
        xpad[:, 1:-1, 1:-1] = x[b]
        for s in range(2):
            cols = np.empty((CIN, 3, 3, Hp, Wp, KI, P), np.float32)
            for dy in range(3):
                for dx in range(3):
                    view = xpad[:, dy:dy + H, dx:dx + W]
                    v4 = view.reshape(CIN, Hp, P, Wp, P)[:, :, 8 * s:8 * s + 8]
                    cols[:, dy, dx] = v4.transpose(0, 1, 3, 2, 4)
            # [27, m, off] -> [(4 o', 27 k), (32 q, 256 m_pad)]
            xc = cols.reshape(27, N, OFF).transpose(0, 2, 1)   # [27, off, m]
            tmp = np.zeros((27, OFF, 256), np.float32)
            tmp[:, :, :N] = xc
            xcol = tmp.reshape(27, 32, 4, 256).transpose(2, 0, 1, 3)
            xcol = xcol.reshape(108, 8192).astype(bf)
            in_maps.append(dict(shared, xcol=xcol, patches=patches))
    return in_maps


def kernel(**inputs):
    if "nc" not in _CACHE:
        _CACHE["nc"] = _build()
    nc = _CACHE["nc"]
    in_maps = _host_prep(inputs)
    res = run_bass_kernel_spmd(nc, in_maps, core_ids=list(range(8)))
    out = np.zeros((B, DIM, H, W), np.float32)
    ov = out.reshape(B, DIM, Hp, P, Wp, P)
    for i, r in enumerate(res.results):
        b, s = divmod(i, 2)
        # out cols = (off, n) = (ki, kj, hp, wp)
        o = np.asarray(r["out"], dtype=np.float32)
        o = o.reshape(DIM, KI, P, Hp, Wp)
        ov[b, :, :, 8 * s:8 * s + 8, :, :] = o.transpose(0, 3, 1, 4, 2)
    return out


# revision 92
# speedup vs baseline: 1.0857x; 1.0323x over previous
"""Trainium2 Bass kernel for nn_Attention_87668872446719.

Patch-attention module: v = Conv3x3(x); xe = PatchEmbed(x); q,k = Linear(xe);
attn = softmax(q k^T / sqrt(hd)); out = Fold(attn @ Unfold(v)); out = Conv1x1(out).

Identity used (validated numerically): the unfold/attn/fold pipeline equals,
per channel c with head h = c // 32:
    folded[c, patch n, off] = sum_m attn[h, n, m] * v[c, patch m, off]

Sharding (8 cores, no collectives): core = (image b in 0..3, half s in 0..1).
s splits every 16x16 patch into its top/bottom 8 rows (off = ki*16+kj with
ki in [8s, 8s+8)), so the 1x1 proj stays pixel-local per core and each core
writes disjoint output rows.

Per core on device (all matmuls bf16, f32 PSUM accumulation):
  1. v conv first (warms the PE), TRANSPOSED output: lhsT = im2col slice
     [27, m-chunk] (pixel cols ordered o-major), rhs = wvT[27, 256] ->
     psum[m, 256 c] per o -> evict into VT[mc] = [m, (o, c)] bf16.
     V never leaves SBUF and needs no partition shuffle.
  2. xeT[256,196] = patch embed; qT/kT[32,196] per head (q pre-scaled)
  3. S[n,m] per head -> softmax; 1/rowsum folded into the bf16 cast of A;
     A transposed to AT[m, n] via PE (chunks of 98)
  4. stage E (F^T form): for each c: psum[off(128), n(196)] accumulated
     over m-chunks with lhsT = VT[:, c::256] (o-strided), rhs = AT
     -> fsb[off, (c, n)] -> fdram[off, c, n] (bf16, 12.5KB contiguous
     writes per off-row; the o<->c scatter cost is paid on the read side
     where it overlaps stage E + proj)
  5. proj: out[oc, (off n)] = projw @ F read back as [c, (32 off, n)]
     tiles; bf16 output (host upcasts to f32)
"""
from contextlib import ExitStack

import numpy as np
import ml_dtypes

import concourse.bass as bass
import concourse.tile as tile
from concourse import bacc, mybir
from concourse.bass_utils import run_bass_kernel_spmd

B, CIN, H, W = 4, 3, 224, 224
P = 16
DIM = 256
HEADS = 8
Hp = Wp = 14
N = Hp * Wp            # 196 patches
HD = DIM // HEADS      # 32
KI = 8                 # patch rows per core
OFF = KI * P           # 128 within-patch pixels per core
NPIX = N * OFF         # 25088 pixels per core
MCH = 98               # m-chunk (2 chunks of 98)
NCH = 98               # n-chunk for softmax/transposes
Q = 49                 # conv m-block (4 blocks of 49 m per cc)
BF = mybir.dt.bfloat16
F32 = mybir.dt.float32
AFT = mybir.ActivationFunctionType
AX = mybir.AxisListType.X

_CACHE = {}


def _build():
    nc = bacc.Bacc("TRN2", target_bir_lowering=False, debug=False)

    xcol_d = nc.declare_dram_parameter("xcol", [108, 8192], BF, isOutput=False)
    patches_d = nc.declare_dram_parameter("patches", [128, 6, N], BF, isOutput=False)
    pwT_d = nc.declare_dram_parameter("pwT", [128, 6, DIM], BF, isOutput=False)
    qkwT_d = nc.declare_dram_parameter("qkwT", [128, 2, 2 * DIM], BF, isOutput=False)
    wvT_d = nc.declare_dram_parameter("wvT", [108, 1024], BF, isOutput=False)
    projwT_d = nc.declare_dram_parameter("projwT", [128, 2, DIM], BF, isOutput=False)
    pbias_d = nc.declare_dram_parameter("pbias", [128, 2], F32, isOutput=False)
    obias_d = nc.declare_dram_parameter("obias", [128, 2], F32, isOutput=False)
    ident_d = nc.declare_dram_parameter("ident", [NCH, NCH], BF, isOutput=False)
    out_d = nc.declare_dram_parameter("out", [DIM, NPIX], BF, isOutput=True)

    fdram = nc.dram_tensor("fdram", [OFF, DIM, N], BF)       # [off, c, n]

    with tile.TileContext(nc) as tc, ExitStack() as ctx:
        const = ctx.enter_context(tc.tile_pool(name="const", bufs=1))
        stat = ctx.enter_context(tc.tile_pool(name="stat", bufs=4))
        sb = ctx.enter_context(tc.tile_pool(name="sb", bufs=2))
        atp = ctx.enter_context(tc.tile_pool(name="atp", bufs=1))
        pP = ctx.enter_context(tc.tile_pool(name="pP", bufs=2, space="PSUM"))
        pA = ctx.enter_context(tc.tile_pool(name="pA", bufs=3, space="PSUM"))
        pfr1 = ctx.enter_context(tc.tile_pool(name="pfr1", bufs=2))
        vctx = ctx.enter_context(ExitStack())
        vtp = vctx.enter_context(tc.tile_pool(name="vtp", bufs=1))
        GW = 32 * N  # 6272 cols per og chunk

        def fr_load_cc1(og):
            fr = pfr1.tile([128, GW], BF, tag="fr1", name="fr")
            src = fdram[og * 32:(og + 1) * 32, 128:256, :].rearrange(
                "o c n -> c o n")
            nc.sync.dma_start(
                fr[:].rearrange("c (o n) -> c o n", n=N), src)
            return fr

        # ---- constants (spread across issue queues; xcol gates conv and
        # is issued first, on its own queue) ----
        qrot = [nc.scalar, nc.sync]

        def cload(shape, dt, dram, tag, qi=[0]):
            t = const.tile(shape, dt, tag=tag, name=tag)
            q = qrot[qi[0] % 2]
            qi[0] += 1
            q.dma_start(t[:], dram[:])
            return t

        with tc.high_priority():
            wvT_t = cload([108, 1024], BF, wvT_d, "c_wvT")
        patches_t = cload([128, 6, N], BF, patches_d, "c_patches")
        pwT_t = cload([128, 6, DIM], BF, pwT_d, "c_pwT")
        qkwT_t = cload([128, 2, 2 * DIM], BF, qkwT_d, "c_qkwT")
        projwT_t = cload([128, 2, DIM], BF, projwT_d, "c_projwT")
        pbias_t = cload([128, 2], F32, pbias_d, "c_pbias")
        obias_t = cload([128, 2], F32, obias_d, "c_obias")
        ident_t = cload([NCH, NCH], BF, ident_d, "c_ident")

        # VT[mc]: [128, (128 off, 256 c)] bf16, partition = m (m padded
        # to 256 with zero rows so stage E runs K=128)
        VT = [vtp.tile([128, OFF * DIM], BF, tag="vt%d" % mc,
                       name="vt%d" % mc) for mc in range(2)]

        ev_flip = [0]

        def evict(dst, src, scale=None, bias=None):
            """PSUM -> SBUF eviction alternating DVE / ACT."""
            e = ev_flip[0] = 1 - ev_flip[0]
            if scale is not None:
                if e:
                    nc.vector.tensor_scalar_mul(dst, src, scale)
                else:
                    nc.scalar.activation(dst, src, AFT.Copy, scale=scale)
            elif bias is not None:
                if e:
                    nc.vector.tensor_scalar_add(dst, src, bias)
                else:
                    nc.scalar.activation(dst, src, AFT.Identity, bias=bias)
            else:
                if e:
                    nc.vector.tensor_copy(dst, src)
                else:
                    nc.scalar.copy(dst, src)

        # ---- stage D first (warms PE early): v conv, transposed out ----
        # K packed to 108 = 4 off x 27 k (HAM duty 84%): lhsT = xcol
        # [108, m-chunk 128] (cols (o-quad, m_pad 256)); rhs = wvT
        # block-diagonal [108, (2 chalf, 4 off, 128 c)]; 2 MMs of N=512
        # per (o-quad, mc) share one LDWEIGHTS.
        with tc.tile_pool(name="px", bufs=1) as px:
            xcol_t = px.tile([108, 8192], BF, tag="xcol", name="xcol")
            with tc.high_priority():
                for i4 in range(4):
                    q = nc.sync if i4 % 2 == 0 else nc.scalar
                    q.dma_start(xcol_t[:, i4 * 2048:(i4 + 1) * 2048],
                                xcol_d[:, i4 * 2048:(i4 + 1) * 2048])
            for o4 in range(32):      # 4 off per psum tile
                for mc in range(2):
                    ps = pA.tile([128, 1024], F32, tag="mm", name="psc")
                    for chalf in range(2):
                        nc.tensor.matmul(
                            ps[:, chalf * 512:(chalf + 1) * 512],
                            xcol_t[:, o4 * 256 + mc * 128:
                                   o4 * 256 + mc * 128 + 128],
                            wvT_t[:, chalf * 512:(chalf + 1) * 512],
                            start=True, stop=True)
                    # psum (chalf, off, c) -> VT (off, chalf, c):
                    # one engine per chalf, in parallel
                    vtv = VT[mc][:, o4 * 1024:(o4 + 1) * 1024].rearrange(
                        "m (o h c) -> m h o c", o=4, h=2)
                    for h2 in range(2):
                        s2 = ps[:, h2 * 512:(h2 + 1) * 512].rearrange(
                            "m (o c) -> m o c", o=4)
                        if h2 == 0:
                            nc.vector.tensor_copy(vtv[:, h2], s2)
                        else:
                            nc.scalar.copy(vtv[:, h2], s2)

        # ---- stage A: xeT[c, n] = patch embed (transposed) ----
        xeT = []
        for cc in range(2):
            ps = pP.tile([128, N], F32, tag="sm", name="pse")
            for kc in range(6):
                nc.tensor.matmul(
                    ps[:], pwT_t[:, kc, cc * 128:(cc + 1) * 128],
                    patches_t[:, kc, :], start=(kc == 0), stop=(kc == 5))
            xt = sb.tile([128, N], BF, tag="xeT%d" % cc, name="xeT")
            nc.vector.tensor_scalar_add(xt[:], ps[:], pbias_t[:, cc:cc + 1])
            xeT.append(xt)

        # ---- stage B/C: per-head q/k, scores, softmax, AT ----
        # (emitted per-head, software-pipelined into stage E)
        AT = {}     # AT[h][mc] : [128, 196] bf16 (A^T, normalized)

        def phase1_head(h):
            qT = sb.tile([HD, N], BF, tag="qT", name="qT")
            kT = sb.tile([HD, N], BF, tag="kT", name="kT")
            for dst, joff in ((qT, h * HD), (kT, DIM + h * HD)):
                ps = pP.tile([HD, N], F32, tag="sm", name="psq")
                for cc in range(2):
                    nc.tensor.matmul(
                        ps[:], qkwT_t[:, cc, joff:joff + HD], xeT[cc][:],
                        start=(cc == 0), stop=(cc == 1))
                nc.scalar.copy(dst[:], ps[:])

            Ah = []
            for nci in range(2):
                nb = nci * NCH
                ps = pP.tile([NCH, N], F32, tag="sm", name="pss")
                nc.tensor.matmul(ps[:], qT[:, nb:nb + NCH], kT[:],
                                 start=True, stop=True)
                mx = stat.tile([NCH, 1], F32, tag="mx", name="mx")
                nc.vector.reduce_max(mx[:], ps[:], axis=AX, negate=True)
                ex = sb.tile([NCH, N], BF, tag="ex", name="ex")
                nc.scalar.activation(ex[:], ps[:], AFT.Exp, bias=mx[:])
                sm = stat.tile([NCH, 1], F32, tag="smm", name="smm")
                nc.vector.reduce_sum(sm[:], ex[:], axis=AX)
                rc = stat.tile([NCH, 1], F32, tag="rc", name="rc")
                nc.vector.reciprocal(rc[:], sm[:])
                ab = sb.tile([NCH, 256], BF, tag="ab", name="ab")
                nc.vector.tensor_scalar_mul(ab[:, :N], ex[:], rc[:])
                nc.vector.memset(ab[:, N:], 0.0)
                Ah.append(ab)

            ATh = []
            for mc in range(2):
                at = atp.tile([128, N], BF, tag="at%d_%d" % (mc, h), name="at")
                mb = mc * 128
                for nci in range(2):
                    nb = nci * NCH
                    pt = pP.tile([128, NCH], BF, tag="sm", name="pst")
                    nc.tensor.transpose(pt[:], Ah[nci][:, mb:mb + 128],
                                        ident_t[:])
                    evict(at[:, nb:nb + NCH], pt[:])
                ATh.append(at)
            AT[h] = ATh

        # ---- stage E (F^T form): psum[off, n] per c, evict to fsb ----
        # VT free layout is (q32, h2, o4, c128); lhsT for global c is a
        # 3D strided slice.  Head order interleaves cc0/cc1 so the
        # scattered fdramA writes get two-head time budgets.
        VTv = [VT[mc].rearrange("m (o c) -> m o c", c=DIM) for mc in range(2)]
        fr1_pend = {}
        HORDER = (4, 5, 6, 7, 0, 1, 2, 3)
        phase1_head(HORDER[0])
        phase1_head(HORDER[1])
        with tc.tile_pool(name="fsp", bufs=2) as fsp:
            for hi, h in enumerate(HORDER):
                if hi + 2 < HEADS:
                    phase1_head(HORDER[hi + 2])
                if hi == 4:
                    # cc1 F rows complete: prefetch cc1 reads under the
                    # cc0 heads' compute
                    fr1_pend[0] = fr_load_cc1(0)
                    fr1_pend[1] = fr_load_cc1(1)
                fsb = fsp.tile([128, 32 * N], BF, tag="fsb", name="fsb")
                for jj in range(8):   # groups of 4 c
                    ps = pA.tile([128, 1024], F32, tag="mm", name="psf")
                    for j2 in range(4):
                        cg = h * 32 + jj * 4 + j2
                        o0 = (j2 // 2) * 512 + (j2 % 2) * N
                        for mc in range(2):
                            nc.tensor.matmul(
                                ps[:, o0:o0 + N],
                                VTv[mc][:, :, cg],
                                AT[h][mc][:],
                                start=(mc == 0), stop=(mc == 1))
                    src = ps[:].rearrange("p (b x) -> p b x", b=2)[:, :, :2 * N]
                    dst = fsb[:, jj * 4 * N:(jj + 1) * 4 * N].rearrange(
                        "p (b x) -> p b x", b=2)
                    evict(dst, src)
                fd = fdram[:, h * HD:(h + 1) * HD, :]
                nc.sync.dma_start(
                    fd, fsb[:].rearrange("o (c n) -> o c n", n=N))

        # ---- stage F: proj from fdram, bf16 out ----
        # fr tiles [c(128), (16 off, 196 n)] per (og16, cc).  pfr lives
        # alongside VT (og16 keeps it small) so the first reads overlap
        # the stage E tail; bufs=2 keeps reads 2 chunks ahead of the
        # matmuls.  Out cols remain (off, n) order.
        vctx.close()   # free VT before proj pools allocate
        pfr0 = ctx.enter_context(tc.tile_pool(name="pfr0", bufs=2))
        posb = ctx.enter_context(tc.tile_pool(name="posb", bufs=3))

        def fr_load_cc0(og):
            fr = pfr0.tile([128, GW], BF, tag="fr0", name="fr")
            src = fdram[og * 32:(og + 1) * 32, 0:128, :].rearrange(
                "o c n -> c o n")
            nc.sync.dma_start(
                fr[:].rearrange("c (o n) -> c o n", n=N), src)
            return fr

        fr0_pend = {0: fr_load_cc0(0), 1: fr_load_cc0(1)}
        for og in range(4):
            if og + 2 < 4:
                fr0_pend[og + 2] = fr_load_cc0(og + 2)
                fr1_pend[og + 2] = fr_load_cc1(og + 2)
            frs = [fr0_pend[og], fr1_pend[og]]
            for occ in range(2):
                ot = posb.tile([128, GW], BF, tag="osb", name="osb")
                for t6 in range(7):
                    w = 1024 if t6 < 6 else 128
                    ps = pA.tile([128, 1024], F32, tag="mm", name="psp")
                    for half in range((w + 511) // 512):
                        b0 = t6 * 1024 + half * 512
                        bw = min(512, w - half * 512)
                        for cc in range(2):
                            nc.tensor.matmul(
                                ps[:, half * 512:half * 512 + bw],
                                projwT_t[:, cc, occ * 128:(occ + 1) * 128],
                                frs[cc][:, b0:b0 + bw],
                                start=(cc == 0), stop=(cc == 1))
                    evict(ot[:, t6 * 1024:t6 * 1024 + w], ps[:, :w],
                          bias=obias_t[:, occ:occ + 1])
                nc.sync.dma_start(
                    out_d[occ * 128:(occ + 1) * 128,
                          og * GW:(og + 1) * GW], ot[:])

    nc.compile()
    return nc


def _host_prep(inputs):
    """Returns per-core in_maps."""
    x = np.asarray(inputs["x"], np.float32)
    patch_w = np.asarray(inputs["patch_w"], np.float32)
    patch_b = np.asarray(inputs["patch_b"], np.float32)
    qk_w = np.asarray(inputs["qk_w"], np.float32)
    v_w = np.asarray(inputs["v_w"], np.float32)
    v_b = np.asarray(inputs["v_b"], np.float32)
    proj_w = np.asarray(inputs["proj_w"], np.float32).reshape(DIM, DIM)
    proj_b = np.asarray(inputs["proj_b"], np.float32)

    bf = ml_dtypes.bfloat16
    pw = patch_w.reshape(DIM, CIN * P * P)                     # [256, 768]
    pwT = pw.T.reshape(6, 128, DIM).transpose(1, 0, 2)         # [128, 6, 256]
    qkw = qk_w.copy()
    qkw[:DIM] *= HD ** -0.5                                    # fold attn scale
    qkwT = qkw.T.reshape(2, 128, 2 * DIM).transpose(1, 0, 2)   # [128, 2, 512]
    wvT = v_w.reshape(DIM, 27).T                               # [27, 256]
    # block-diagonal over 4 off-slices: [(4 o', 27 k), (2 ch, 4 o'', 128 c)]
    wvT4 = np.zeros((108, 1024), np.float32)
    for op in range(4):
        for ch in range(2):
            wvT4[op * 27:(op + 1) * 27,
                 ch * 512 + op * 128:ch * 512 + op * 128 + 128] = \
                wvT[:, ch * 128:(ch + 1) * 128]
    projwT = proj_w.T.reshape(2, 128, DIM).transpose(1, 0, 2)  # [128, 2, 256]
    pbias = patch_b.reshape(2, 128).T.copy()                   # [128, 2]
    obias = (proj_w @ v_b + proj_b).reshape(2, 128).T.copy()   # [128, 2]

    shared = {
        "pwT": pwT.astype(bf), "qkwT": qkwT.astype(bf),
        "wvT": wvT4.astype(bf), "projwT": projwT.astype(bf),
        "pbias": pbias.astype(np.float32), "obias": obias.astype(np.float32),
        "ident": np.eye(NCH, dtype=bf),
    }

    in_maps = []
    for b in range(B):
        # patches: [768, 196] part order (ci, ki, kj) -> [128, 6, 196]
        p4 = x[b].reshape(CIN, Hp, P, Wp, P).transpose(0, 2, 4, 1, 3)
        patches = p4.reshape(CIN * P * P, N).reshape(6, 128, N)
        patches = patches.transpose(1, 0, 2).astype(bf)
        xpad = np.zeros((CIN, H + 2, W + 2), np.float32)
        xpad[:, 1:-1, 1:-1] = x[b]
        for s in range(2):
            cols = np.empty((CIN, 3, 3, Hp, Wp, KI, P), np.float32)
            for dy in range(3):
                for dx in range(3):
                    view = xpad[:, dy:dy + H, dx:dx + W]
                    v4 = view.reshape(CIN, Hp, P, Wp, P)[:, :, 8 * s:8 * s + 8]
                    cols[:, dy, dx] = v4.transpose(0, 1, 3, 2, 4)
            # [27, m, off] -> [(4 o', 27 k), (32 q, 256 m_pad)]
            xc = cols.reshape(27, N, OFF).transpose(0, 2, 1)   # [27, off, m]
            tmp = np.zeros((27, OFF, 256), np.float32)
            tmp[:, :, :N] = xc
            xcol = tmp.reshape(27, 32, 4, 256).transpose(2, 0, 1, 3)
            xcol = xcol.reshape(108, 8192).astype(bf)
            in_maps.append(dict(shared, xcol=xcol, patches=patches))
    return in_maps


def kernel(**inputs):
    if "nc" not in _CACHE:
        _CACHE["nc"] = _build()
    nc = _CACHE["nc"]
    in_maps = _host_prep(inputs)
    res = run_bass_kernel_spmd(nc, in_maps, core_ids=list(range(8)))
    out = np.zeros((B, DIM, H, W), np.float32)
    ov = out.reshape(B, DIM, Hp, P, Wp, P)
    for i, r in enumerate(res.results):
        b, s = divmod(i, 2)
        # out cols = (off, n) = (ki, kj, hp, wp)
        o = np.asarray(r["out"], dtype=np.float32)
        o = o.reshape(DIM, KI, P, Hp, Wp)
        ov[b, :, :, 8 * s:8 * s + 8, :, :] = o.transpose(0, 3, 1, 4, 2)
    return out
